# revision 28
# baseline (speedup 1.0000x reference)
"""Multi-head causal attention with RoPE on 8 Trainium2 NeuronCores.

Problem: B=2, S=2048, D=1024, H=16 heads (dk=64), fp32 in/out, causal mask,
RoPE on Q/K, y = softmax(QK^T/sqrt(dk)) V projected by Wo.

Sharding: head-parallel. Core c owns 2 heads (columns c*128:(c+1)*128 of the
QKV projection output). All matmul operands are bf16 (fp32 PSUM accumulate).

Schedule (trace-driven; see the per-version notes in the session log):
  1. The CC-core performs a one-time ~35us setup at a fixed ~[21,57]us
     that freezes DMA progress. Everything needed before ~57us (chunks
     0-3, weights, rope/mask tables) is DMA'd first and lands by ~20us;
     later chunks + Wo ride out the freeze on the sync queue ONLY - a
     stalled DMA-issue on the scalar engine queue would head-of-line
     block the softmax exp ACTs behind it (strict FIFO).
  2. Dependency-free scratch matmuls at t~0 warm the PE HAM clock; the
     dummy warm-up AllToAll is triggered immediately at kernel start.
  3. Attention is ONE software-pipelined stream per batch over all
     (q-chunk, k-tile) tiles: at each q-chunk boundary the next chunk's
     first score tile is emitted before the previous chunk's l-block, and
     filler work slots in there - QKV chunks 2-5 fill batch-0's
     attention, chunks 6-7 + the AllToAll(b0) unpack fill batch-1's, the
     batch-0 projection covers the AllToAll(b1) window, and dummy-matmul
     bridges span the remaining collective waits so the PE never idles
     into a HAM re-throttle (a >3.4us PE lull halves the clock for
     10-27us).
  4. softmax denominators: l sits in row 64 of the PV PSUM tile (ones
     column trick); PE-transposed to [128, 8] for one cheap DVE
     reciprocal, transposed back, broadcast with a K=1 ones matmul; the
     final scale runs on DVE straight from PSUM.
  5. causal attention with transposed scores ST[k,q]: exp on ScalarE
     straight out of PSUM (both heads in one strided ACT), diagonal
     k-tiles only compute the valid q-range + a static triangle mask on
     GpSimd; score matmuls for the two heads run concurrently on
     disjoint PE row-groups (base partitions 0/64).
  6. AllToAll(b0) triggers right after batch-0 staging and completes
     under batch-1 attention; only AllToAll(b1) + the final projection
     remain in the tail.
Core c owns tokens [c*256,(c+1)*256) of each batch; host reassembles.
"""

import sys

for p in ("/opt/trn_rl_repo", "/root/.axon_site/_ro/trn_rl_repo"):
    if p not in sys.path:
        sys.path.insert(0, p)

import math

import numpy as np
import ml_dtypes

import concourse.bass as bass
import concourse.tile as tile
from concourse import mybir
from concourse.bass_utils import run_bass_kernel_spmd

N_CORES = 8
B, S, D, H = 2, 2048, 1024, 16
DK = D // H          # 64
HPC = H // N_CORES   # heads per core = 2
FW = HPC * DK        # head-group width per core = 128
T = B * S            # 4096 flattened tokens
TCH = 512            # token chunk for projections
NCH = T // TCH       # 8 chunks
KT = 128             # k tile
QC = 512             # q chunk in attention
TPB = S // N_CORES   # 256 output tokens per core per batch
TSL = B * TPB        # 512 output rows per core

F32 = mybir.dt.float32
BF16 = mybir.dt.bfloat16
BF = np.dtype(ml_dtypes.bfloat16)


def _spill_waits(nc, max_other=1):
    """walrus in this container allows 1 sync-wait per instruction; move
    excess waits onto preceding single-wait NoOps on the same engine."""
    n_new = 0
    for bb in nc.m.functions[0].blocks:
        newlist = []
        changed = False
        for inst in bb.instructions:
            si = inst.sync_info
            if si is not None and si.on_wait and len(si.on_wait) > max_other:
                waits = list(si.on_wait)
                overflow, keep = waits[:-max_other], waits[-max_other:]
                while overflow:
                    chunk, overflow = overflow[:1], overflow[1:]
                    nop = mybir.InstNoOp(
                        name=f"waitspill{n_new}-{inst.name}", ins=[], outs=[]
                    )
                    nop.engine = inst.engine
                    nop.debug = inst.debug
                    nop.sync_info = mybir.SyncInfo(on_wait=chunk, on_update=[])
                    newlist.append(nop)
                    n_new += 1
                si.on_wait = keep
                inst.sync_info = si
                changed = True
            newlist.append(inst)
        if changed:
            bb.instructions = newlist
    return n_new


def build_kernel():
    nc = bass.Bass("TRN2", num_devices=N_CORES)

    x5 = nc.dram_tensor("x5", [NCH, 128, 8, TCH], BF16, kind="ExternalInput")
    wq = nc.dram_tensor("wq", [128, 8, FW], BF16, kind="ExternalInput")  # pre-scaled
    wk = nc.dram_tensor("wk", [128, 8, FW], BF16, kind="ExternalInput")
    wv = nc.dram_tensor("wv", [128, 8, FW], BF16, kind="ExternalInput")
    woT = nc.dram_tensor("woT", [D, D], BF16, kind="ExternalInput")
    ctab = nc.dram_tensor("ctab", [FW, S], BF16, kind="ExternalInput")
    stab = nc.dram_tensor("stab", [FW, S], BF16, kind="ExternalInput")
    tri = nc.dram_tensor("tri", [KT, KT], BF16, kind="ExternalInput")
    ident = nc.dram_tensor("ident", [128, 128], BF16, kind="ExternalInput")
    perm = nc.dram_tensor("perm", [128, 128], BF16, kind="ExternalInput")
    y = nc.dram_tensor("y", [TSL, D], F32, kind="ExternalOutput")

    with tile.TileContext(nc) as tc:
        with (
            tc.tile_pool(name="warm", bufs=1) as warmpool,
            tc.tile_pool(name="const", bufs=1) as const,
            tc.tile_pool(name="xch", bufs=8) as xch,
            tc.tile_pool(name="qk", bufs=1) as qkpool,
            tc.tile_pool(name="tmp", bufs=3) as tmp,
            tc.tile_pool(name="pts", bufs=4) as pts,
            tc.tile_pool(name="lpool", bufs=2) as lpool,
            tc.tile_pool(name="wo", bufs=8) as wopool,
            tc.tile_pool(name="yout", bufs=2) as ypool,
            tc.tile_pool(name="mm", bufs=2, space="PSUM") as mmps,
            tc.tile_pool(name="st", bufs=2, space="PSUM") as stps,
            tc.tile_pool(name="pv", bufs=2, space="PSUM") as pvps,
            tc.tile_pool(name="dram", bufs=1, space="DRAM") as dram,
        ):
            warm_in = dram.tile([8, 16], F32)
            warm_out = dram.tile([8, 16], F32)

            # warm-up collective FIRST: the CC-core does a one-time ~35us
            # setup that freezes the HWDGE rings (observed ~[21us, 57us]);
            # triggering the dummy AllToAll immediately keeps that window as
            # early as possible and off the gpsimd queue's critical path
            wtile = warmpool.tile([1, 128], F32)
            nc.vector.memset(wtile, 0.0)
            nc.gpsimd.dma_start(
                out=warm_in[:, :],
                in_=wtile[:1, :128].rearrange("p (a f) -> (p a) f", a=8),
            )
            nc.gpsimd.collective_compute(
                "AllToAll",
                mybir.AluOpType.bypass,
                replica_groups=[list(range(N_CORES))],
                ins=[warm_in[:].opt()],
                outs=[warm_out[:].opt()],
            )

            # warm the PE HAM clock while the first input DMAs are still in
            # flight: dependency-free matmuls on an uninitialized scratch
            # tile (output never consumed) so the real QKV starts at 2.4GHz
            scratch = warmpool.tile([128, TCH], BF16)
            nc.vector.memset(scratch, 1.0)
            warm0 = mmps.tile([128, TCH], F32, tag="mm", name="warm0")
            for i in range(16):
                nc.tensor.matmul(
                    warm0, scratch[:, 0:128], scratch,
                    start=(i == 0), stop=(i == 15),
                )

            # ---- all input DMAs up front, dependency-priority order ----
            # sync queue: wq, xc0a, rope tables, then remaining chunk a-halves
            # scalar queue: xc0b, wk, wv, perm/ident, chunk b-halves, tri, wo
            wq_sb = const.tile([128, 8, FW], BF16)
            wk_sb = const.tile([128, 8, FW], BF16)
            wv_sb = const.tile([128, 8, FW], BF16)
            c_sb = const.tile([FW, S], BF16)
            s_sb = const.tile([FW, S], BF16)
            tri_sb = const.tile([KT, KT], BF16)
            id_sb = const.tile([128, 128], BF16)
            pm_sb = const.tile([128, 128], BF16)

            xtiles = {}

            def load_chunk_a(ci):
                xc = xch.tile([128, 8, TCH], BF16, tag="x", name="xc")
                nc.sync.dma_start(out=xc[:, 0:4, :], in_=x5[ci, :, 0:4, :])
                xtiles[ci] = xc

            def load_chunk_b(ci, eng=None):
                # chunks whose issue would still be queued when the CC-boot
                # ring freeze hits must NOT sit on the scalar engine queue:
                # the stalled issue head-of-line-blocks the exp ACTs behind
                # it. Late chunks go on sync instead.
                (eng or nc.scalar).dma_start(
                    out=xtiles[ci][:, 4:8, :], in_=x5[ci, :, 4:8, :]
                )

            nc.sync.dma_start(out=wq_sb, in_=wq[:, :, :])
            load_chunk_a(0)
            load_chunk_b(0)
            nc.scalar.dma_start(out=wk_sb, in_=wk[:, :, :])
            nc.scalar.dma_start(out=wv_sb, in_=wv[:, :, :])
            nc.sync.dma_start(out=c_sb, in_=ctab[:, :])
            nc.sync.dma_start(out=s_sb, in_=stab[:, :])
            nc.scalar.dma_start(out=pm_sb, in_=perm[:, :])
            nc.scalar.dma_start(out=id_sb, in_=ident[:, :])
            nc.scalar.dma_start(out=tri_sb, in_=tri[:, :])
            # chunks 0-3 + all tables land before the ~21us CC-boot ring
            # freeze; later chunks resume after it, each well before its
            # interleaved QKV slot needs it
            for ci in range(1, 4):
                load_chunk_a(ci)
                load_chunk_b(ci)
            for ci in range(4, NCH):
                load_chunk_a(ci)
                load_chunk_b(ci, eng=nc.sync)
            wo_sb = []
            for p in range(N_CORES):
                wt = wopool.tile([128, D], BF16, tag="wo")
                nc.sync.dma_start(out=wt, in_=woT[p * 128 : (p + 1) * 128, :])
                wo_sb.append(wt)

            ones_f = const.tile([1, DK], F32)
            nc.vector.memset(ones_f, 1.0)
            ones64 = const.tile([1, DK], BF16)
            nc.vector.tensor_copy(out=ones64, in_=ones_f)

            qT = qkpool.tile([FW, T], BF16, tag="qT")
            kTt = qkpool.tile([FW, T], BF16, tag="kT")
            v_sb = qkpool.tile([128, T // 128, 2 * DK + 2], BF16, tag="v")
            outT = qkpool.tile([FW, T], BF16, tag="outT")
            # bake the ones columns for the softmax denominator
            vones = const.tile([128, T // 128], F32)
            nc.vector.memset(vones, 1.0)
            nc.vector.tensor_copy(out=v_sb[:, :, DK], in_=vones)
            nc.vector.tensor_copy(out=v_sb[:, :, 2 * DK + 1], in_=vones)

            # ---- QKV projections + RoPE (per x chunk) ----
            def do_qkv_chunk(ci):
                t0 = ci * TCH
                sc = (ci % (S // TCH)) * TCH  # position within batch for rope
                xc = xtiles.pop(ci)

                # Q accumulates in "mm", K in "st" so the two groups never
                # stall on the same PSUM slot rotation.
                for which, w_sb, dst in (("q", wq_sb, qT), ("k", wk_sb, kTt)):
                    if which == "q":
                        ps = mmps.tile([FW, TCH], F32, tag="mm", name="qps")
                    else:
                        ks = stps.tile([128, 2, TCH], F32, tag="st", name="kps")
                        ps = ks[:, 0, :]
                    for dt in range(8):
                        nc.tensor.matmul(
                            ps,
                            w_sb[:, dt, :],
                            xc[:, dt, :],
                            start=(dt == 0),
                            stop=(dt == 7),
                        )
                    raw = tmp.tile([FW, TCH], BF16, tag="raw", name="raw")
                    nc.vector.tensor_copy(out=raw, in_=ps)
                    # pair swap across partitions via a PE permutation matmul
                    swp = pvps.tile([FW, TCH], F32, tag="pv", name="swp")
                    nc.tensor.matmul(swp, pm_sb, raw, start=True, stop=True)
                    dslice = dst[:, t0 : t0 + TCH]
                    nc.vector.tensor_mul(dslice, raw, c_sb[:, sc : sc + TCH])
                    t2 = tmp.tile([FW, TCH], BF16, tag="ropetmp", name="t2")
                    nc.vector.tensor_mul(t2, swp, s_sb[:, sc : sc + TCH])
                    nc.vector.tensor_add(dslice, dslice, t2)

                # V^T [FW, TCH] like Q/K, then PE identity transpose into v_sb
                vps = mmps.tile([FW, TCH], F32, tag="mm", name="vps")
                for dt in range(8):
                    nc.tensor.matmul(
                        vps,
                        wv_sb[:, dt, :],
                        xc[:, dt, :],
                        start=(dt == 0),
                        stop=(dt == 7),
                    )
                vstage = tmp.tile([FW, TCH], BF16, tag="vstage", name="vstage")
                nc.scalar.copy(out=vstage, in_=vps)
                vtp = mmps.tile([128, TCH // 128, 128], BF16, tag="mm", name="vtp")
                c0 = t0 // 128
                for i in range(TCH // 128):
                    nc.tensor.transpose(
                        vtp[:, i, :], vstage[:, i * 128 : (i + 1) * 128], id_sb
                    )
                    vview = v_sb[:, c0 + i, :].rearrange(
                        "p (g j) -> p g j", j=DK + 1
                    )[:, :, 0:DK]
                    nc.scalar.copy(
                        out=vview,
                        in_=vtp[:, i, :].rearrange("p (g j) -> p g j", j=DK),
                    )

            # ---- causal attention for one batch ----
            # Transposed-scores flash style, software-pipelined one k-tile
            # ahead. Diagonal k-tiles only compute the valid q-range.
            def emit_st(b, qc, kt):
                trow = b * S + qc * QC
                kcol = b * S + kt * KT
                o = max(0, (kt - 4 * qc) * KT)
                w = QC - o
                st = stps.tile([128, 2, QC], F32, tag="st", name="st")
                for h2 in range(HPC):
                    fb = h2 * DK
                    nc.tensor.matmul(
                        st[:, h2, 0:w],
                        kTt[fb : fb + DK, kcol : kcol + KT],
                        qT[fb : fb + DK, trow + o : trow + QC],
                        start=True,
                        stop=True,
                    )
                pt = pts.tile([128, 2, QC], BF16, tag="pt", name="pt")
                nc.scalar.activation(
                    out=pt[:, :, 0:w], in_=st[:, :, 0:w],
                    func=mybir.ActivationFunctionType.Exp,
                )
                if kt >= 4 * qc:  # diagonal: mask the triangular 128-col block
                    for h2 in range(HPC):
                        nc.gpsimd.tensor_mul(
                            pt[:, h2, 0:KT], pt[:, h2, 0:KT], tri_sb
                        )
                return pt, o, w

            def emit_pv(b, qc, kt, pv2, prev):
                pt, o, w = prev
                nkt = 4 * (qc + 1)
                for h2 in range(HPC):
                    vcol = h2 * (DK + 1)
                    nc.tensor.matmul(
                        pv2[h2][:, o : o + w],
                        v_sb[:, b * (S // 128) + kt, vcol : vcol + DK + 1],
                        pt[:, h2, 0:w],
                        start=(kt == 0),
                        stop=(kt == nkt - 1),
                        skip_group_check=True,
                    )

            cc_ins = {}

            def finalize_qc(b, qc, pv2):
                    trow = b * S + qc * QC
                    # normalization: evacuate pv to SBUF bf16 right away
                    # (frees the PSUM bank). The softmax denominators l sit as
                    # [1, 512] rows, where any elementwise op runs on a single
                    # lane; PE-transpose them into [128, 2x4] so one cheap
                    # reciprocal covers the whole q-chunk, transpose back, and
                    # broadcast across partitions with a K=1 ones matmul.
                    pvcs = []
                    lT = mmps.tile([128, HPC, 4, 2], BF16, tag="mm", name="lT")
                    for h2 in range(HPC):
                        pvc = lpool.tile([DK + 1, QC], BF16, tag=f"pvc{h2}", name="pvc")
                        # split the two evacuations across DVE and ScalarE so
                        # they run in parallel and the l-chain starts sooner
                        if h2 == 0:
                            nc.vector.tensor_copy(out=pvc, in_=pv2[h2])
                        else:
                            nc.scalar.copy(out=pvc, in_=pv2[h2])
                        pvcs.append(pvc)
                        for blk in range(4):
                            nc.tensor.transpose(
                                lT[:, h2, blk, 0:1],
                                pvc[DK : DK + 1, blk * 128 : (blk + 1) * 128],
                                id_sb[DK : DK + 1, DK : DK + 1],
                            )
                    linvT = lpool.tile([128, HPC, 4], BF16, tag="linvT", name="linvT")
                    with nc.allow_low_precision(reason="bf16 1/l"):
                        nc.vector.reciprocal(out=linvT, in_=lT[:, :, :, 0])
                    for h2 in range(HPC):
                        fb = h2 * DK
                        linvR = mmps.tile([1, 4, 128], BF16, tag="mm", name="linvR")
                        for blk in range(4):
                            nc.tensor.transpose(
                                linvR[:, blk, :],
                                linvT[:, h2, blk : blk + 1],
                                id_sb,
                            )
                        linv_sb = lpool.tile([1, QC], BF16, tag="linv", name="linv_sb")
                        nc.vector.tensor_copy(
                            out=linv_sb, in_=linvR.rearrange("p a f -> p (a f)")
                        )
                        lb = mmps.tile([DK, QC], F32, tag="mm", name="lb")
                        nc.tensor.matmul(lb, ones64, linv_sb, start=True, stop=True)
                        # final scale on DVE straight from PSUM (~0.7us vs
                        # 1.15us on GpSimd, and skips the lb->sbuf copy)
                        nc.vector.tensor_mul(
                            outT[fb : fb + DK, trow : trow + QC],
                            pvcs[h2][0:DK, :],
                            lb,
                        )
                    # stage this q-chunk's collective inputs now so only the
                    # AllToAll itself remains in the tail (HWDGE, sync queue)
                    if b in cc_ins:
                        cc_in = cc_ins[b]
                        for p in (2 * qc, 2 * qc + 1):
                            nc.sync.dma_start(
                                out=cc_in[p, :, :],
                                in_=outT[:, b * S + p * TPB : b * S + (p + 1) * TPB],
                            )

            def attn_batch(b, fillers):
                # One software-pipelined stream over all k-tiles of the batch.
                # At each q-chunk boundary the NEXT chunk's first score tile
                # (and its exp) is emitted before the previous chunk's
                # normalization, so ScalarE stays fed through the l-block;
                # filler work (batch-1 QKV / batch-0 proj / PE bridges) lands
                # there too, keeping the PE out of a HAM re-throttle.
                pv2_cur = None
                prev = None
                for qc in range(4):
                    nkt = 4 * (qc + 1)
                    pv2_new = [
                        pvps.tile([DK + 1, QC], F32, tag="pv", name=f"pv{h2}")
                        for h2 in range(HPC)
                    ]
                    for kt in range(nkt):
                        cur = emit_st(b, qc, kt)
                        if kt >= 1:
                            emit_pv(b, qc, kt - 1, pv2_new, prev)
                        elif qc > 0:
                            emit_pv(b, qc - 1, 4 * qc - 1, pv2_cur, prev)
                            if fillers:
                                fillers.pop(0)()
                            finalize_qc(b, qc - 1, pv2_cur)
                        prev = cur
                    pv2_cur = pv2_new
                # drain: finalize FIRST so the staging DMA (and thus the
                # AllToAll trigger) fires promptly; the tail filler's PE
                # work then runs under the collective
                emit_pv(b, 3, 15, pv2_cur, prev)
                finalize_qc(b, 3, pv2_cur)
                if fillers:
                    fillers.pop(0)()

            # ---- output projection, one 128-token block at a time ----
            def do_proj_tt(half, orecv, tt):
                ysb = ypool.tile([128, D], F32, tag="y")
                for ec in range(D // 512):
                    yps = mmps.tile([128, 512], F32, tag="mm")
                    for p in range(N_CORES):
                        nc.tensor.matmul(
                            yps,
                            orecv[:, p, tt * 128 : (tt + 1) * 128],
                            wo_sb[p][:, ec * 512 : (ec + 1) * 512],
                            start=(p == 0),
                            stop=(p == N_CORES - 1),
                        )
                    nc.vector.tensor_copy(
                        out=ysb[:, ec * 512 : (ec + 1) * 512], in_=yps
                    )
                r0 = half * TPB + tt * 128
                nc.sync.dma_start(out=y[r0 : r0 + 128, :], in_=ysb)

            def a2a_trigger(cc_in, cc_out):
                nc.gpsimd.collective_compute(
                    "AllToAll",
                    mybir.AluOpType.bypass,
                    replica_groups=[list(range(N_CORES))],
                    ins=[cc_in[:].opt()],
                    outs=[cc_out[:].opt()],
                )

            def unpack(cc_out, orecv, use_scalar=False):
                # sync queue by default: it carries nothing latency-critical,
                # so a wait on the collective here can't head-of-line-block
                # the exp ACTs (scalar) or the mask/scale muls (gpsimd)
                for p in range(4):
                    nc.sync.dma_start(out=orecv[:, p, :], in_=cc_out[p, :, :])
                eng = nc.scalar if use_scalar else nc.sync
                for p in range(4, 8):
                    eng.dma_start(out=orecv[:, p, :], in_=cc_out[p, :, :])

            cc_in0 = dram.tile([N_CORES, FW, TPB], BF16)
            cc_out0 = dram.tile([N_CORES, FW, TPB], BF16)
            cc_in1 = dram.tile([N_CORES, FW, TPB], BF16)
            cc_out1 = dram.tile([N_CORES, FW, TPB], BF16)
            orecv0 = qkpool.tile([128, N_CORES, TPB], BF16, tag="or0")
            orecv1 = qkpool.tile([128, N_CORES, TPB], BF16, tag="or1")

            def bridge(n=4):
                # a few dummy matmuls to span a PE lull so HAM stays warm
                warmps = mmps.tile([FW, TCH], F32, tag="mm", name="warm")
                for i in range(n):
                    nc.tensor.matmul(
                        warmps, wq_sb[:, i % 8, :], c_sb[:, 0:TCH],
                        start=(i == 0), stop=(i == n - 1),
                    )

            # ---- two-chunk QKV prologue; chunks 2-7 ride inside the
            # attention streams so the DMA prefetch never has to outrun the
            # CC-boot ring freeze and the PE stream stays dense throughout
            do_qkv_chunk(0)
            do_qkv_chunk(1)

            cc_ins[0] = cc_in0
            attn_batch(0, [lambda ci=ci: do_qkv_chunk(ci) for ci in range(2, 6)])
            a2a_trigger(cc_in0, cc_out0)

            cc_ins[1] = cc_in1
            attn_batch(
                1,
                [
                    lambda: (do_qkv_chunk(6), unpack(cc_out0, orecv0), bridge(8)),
                    lambda: (do_qkv_chunk(7), bridge(8)),
                    lambda: bridge(),
                    lambda: do_proj_tt(0, orecv0, 0),
                ],
            )
            a2a_trigger(cc_in1, cc_out1)
            do_proj_tt(0, orecv0, 1)
            bridge(50)
            unpack(cc_out1, orecv1, use_scalar=True)
            bridge(50)
            do_proj_tt(1, orecv1, 0)
            do_proj_tt(1, orecv1, 1)

    _spill_waits(nc)
    return nc


_NC_CACHE = None


def _get_nc():
    global _NC_CACHE
    if _NC_CACHE is None:
        _NC_CACHE = build_kernel()
    return _NC_CACHE


def _host_prep(x, Wq, Wk, Wv, Wo, token_positions):
    xT = np.ascontiguousarray(x.reshape(T, D).T)  # [D, T]
    # x5[ci, p, dt, t] = xT[dt*128 + p, ci*TCH + t]
    x5 = np.ascontiguousarray(
        xT.reshape(8, 128, NCH, TCH).transpose(2, 1, 0, 3)
    ).astype(BF)
    WqT = Wq.T * np.float32(1.0 / math.sqrt(DK))
    WkT = Wk.T
    WvT = Wv.T
    WoT = np.ascontiguousarray(Wo.T).astype(BF)

    pos = token_positions.astype(np.float64)  # [S]
    i = (np.arange(FW) % DK) // 2  # pair index per row
    inv_freq = 1.0 / (10000.0 ** (2.0 * i / DK))  # [FW]
    ang = inv_freq[:, None] * pos[None, :]  # [FW, S]
    ctab = np.cos(ang).astype(BF)
    sgn = np.where(np.arange(FW) % 2 == 0, -1.0, 1.0)
    stab = (np.sin(ang) * sgn[:, None]).astype(BF)

    tri = (np.arange(KT)[None, :] >= np.arange(KT)[:, None]).astype(BF)
    ident = np.eye(128).astype(BF)
    pr = np.arange(128) ^ 1
    perm = np.zeros((128, 128), np.float32)
    perm[pr, np.arange(128)] = 1.0
    perm = perm.astype(BF)
    return x5, WqT, WkT, WvT, WoT, ctab, stab, tri, ident, perm


def _wslice(WT, c):
    # [D, FW] -> [128, 8, FW] with d = dt*128 + p
    ws = WT[:, c * FW : (c + 1) * FW]
    return np.ascontiguousarray(ws.reshape(8, 128, FW).transpose(1, 0, 2)).astype(BF)


def kernel(x, Wq, Wk, Wv, Wo, mask, token_positions, num_heads, **run_kw):
    x = np.asarray(x)
    assert int(num_heads) == H and x.shape == (B, S, D)
    x5, WqT, WkT, WvT, WoT, ctab, stab, tri, ident, perm = _host_prep(
        np.asarray(x, np.float32),
        np.asarray(Wq, np.float32),
        np.asarray(Wk, np.float32),
        np.asarray(Wv, np.float32),
        np.asarray(Wo, np.float32),
        np.asarray(token_positions),
    )
    in_maps = []
    for c in range(N_CORES):
        in_maps.append(
            {
                "x5": x5,
                "wq": _wslice(WqT, c),
                "wk": _wslice(WkT, c),
                "wv": _wslice(WvT, c),
                "woT": WoT,
                "ctab": ctab,
                "stab": stab,
                "tri": tri,
                "ident": ident,
                "perm": perm,
            }
        )
    nc = _get_nc()
    res = run_bass_kernel_spmd(
        nc, in_maps, core_ids=list(range(N_CORES)), **run_kw
    )
    out = np.empty((B, S, D), dtype=np.float32)
    for c in range(N_CORES):
        yc = res.results[c]["y"]
        out[0, c * TPB : (c + 1) * TPB, :] = yc[0:TPB]
        out[1, c * TPB : (c + 1) * TPB, :] = yc[TPB:TSL]
    kernel.last_results = res
    return out
